# revision 7
# baseline (speedup 1.0000x reference)
"""Multi-head attention (B=4, S=1024, D=1024, H=16) on 8 Trainium2 NeuronCores.

Sharding: core c handles batch b = c//2 and query-half q = c%2 (512 query
rows).  Each core computes K/V projections for its batch's full sequence
(duplicated across the 2 cores sharing a batch), Q projection + attention +
output projection for its 512 query rows.  The full output is a pure
concatenation of the per-core outputs — no collectives needed.

On-chip layout keeps feature dims on SBUF partitions and token dims on the
free axis ("transposed" activations):
  - projections are matmuls with pre-transposed weights as the stationary
    operand, producing X.T layouts directly;
  - attention scores come out as S^T [k, q] (k on partitions) so the AV
    matmul needs no transposes at all;
  - softmax denominators come from an extra all-ones column appended to V
    (AV matmul with M=65: rows 0..63 = numerator, row 64 = denominator);
  - softmax is exp(score/8)*mask with no max subtraction (scores are O(1),
    and exp(-65500) == 0 exactly in fp32, so this matches the reference).
"""

import numpy as np
import ml_dtypes

import concourse.bass as bass
import concourse.tile as tile
from concourse import bacc, mybir
from concourse import bass_utils

B, S, D, H, DK = 4, 1024, 1024, 16, 64
SQ = S // 2            # query rows per core
NT = D // 128          # 8 partition tiles of the feature dim
NCORES = 8
NPAIR = H // 2         # head pairs (2 heads per 128-partition tile)
BF16 = mybir.dt.bfloat16
F32 = mybir.dt.float32
Exp = mybir.ActivationFunctionType.Exp
Identity = mybir.ActivationFunctionType.Identity

_COMPILED = None
TRACE = False
TRACE_CORES = [0]
LAST_RESULT = None


def _emit(nc, tc):
    dram = {n: nc.dram_tensor(n, shp, dt, kind="ExternalInput") for n, shp, dt in [
        ("qT", (NT, 128, SQ), BF16),
        ("kT", (NT, 128, S), BF16),
        ("vT", (NT, 128, S), BF16),
        ("mT", (NT, 128, SQ), BF16),
        ("wq", (NT, 128, D), BF16),
        ("wk", (NT, 128, D), BF16),
        ("wv", (NT, 128, D), BF16),
        ("wo", (NT, 128, D), BF16),
        ("bqc", (128, NT), F32),
        ("bkc", (128, NT), F32),
        ("boc", (128, NT), F32),
        ("bvr", (1, D), BF16),
    ]}
    outT = nc.dram_tensor("outT", (NT, 128, SQ), F32, kind="ExternalOutput")

    import contextlib
    stack = contextlib.ExitStack()
    with stack:
        wpool = stack.enter_context(tc.tile_pool(name="wpool", bufs=1))
        inpool = stack.enter_context(tc.tile_pool(name="inpool", bufs=1))
        acts = stack.enter_context(tc.tile_pool(name="acts", bufs=1))
        xpool = stack.enter_context(tc.tile_pool(name="xpool", bufs=1))
        small = stack.enter_context(tc.tile_pool(name="small", bufs=1))
        opool = stack.enter_context(tc.tile_pool(name="opool", bufs=1))
        psA = stack.enter_context(tc.tile_pool(name="psA", bufs=2, space="PSUM"))
        psB = stack.enter_context(tc.tile_pool(name="psB", bufs=4, space="PSUM"))

        # ---- persistent SBUF tiles ----
        qs = [acts.tile([128, SQ], BF16, name=f"qs{t}") for t in range(NT)]
        ks = [acts.tile([128, S], BF16, name=f"ks{t}") for t in range(NT)]
        vsb = [acts.tile([128, H, DK + 1], BF16, name=f"vsb{t}") for t in range(NT)]
        attnT = [acts.tile([128, SQ], BF16, name=f"attnT{t}") for t in range(NT)]
        mts = [acts.tile([128, SQ], BF16, name=f"mts{t}") for t in range(NT)]

        bq_sb = small.tile([128, NT], F32, name="bq_sb")
        bk_sb = small.tile([128, NT], F32, name="bk_sb")
        bo_sb = small.tile([128, NT], F32, name="bo_sb")
        bv_sb = small.tile([1, D], BF16, name="bv_sb")
        ones1 = small.tile([1, 128], BF16, name="ones1")

        nc.vector.memset(ones1[:], 1.0)
        for t in range(NT):
            nc.vector.memset(vsb[t][:, :, DK:DK + 1], 1.0)

        # ---- constant / bias loads ----
        nc.sync.dma_start(bq_sb[:], dram["bqc"].ap())
        nc.sync.dma_start(bk_sb[:], dram["bkc"].ap())
        nc.sync.dma_start(bo_sb[:], dram["boc"].ap())
        nc.sync.dma_start(bv_sb[:], dram["bvr"].ap())
        for t in range(NT):
            mt_t = mts[t]
            nc.sync.dma_start(mt_t[:], dram["mT"].ap()[t])

        # ---- input/weight tiles (rotating pools) ----
        def load_set(dname, n_free):
            tiles = []
            for t in range(NT):
                tl = inpool.tile([128, S], BF16, tag="in_rot", bufs=16,
                                 name=f"{dname}{t}")
                nc.sync.dma_start(tl[:, :n_free], dram[dname].ap()[t])
                tiles.append(tl)
            return tiles

        def load_w(dname):
            tiles = []
            for t in range(NT):
                tl = wpool.tile([128, D], BF16, tag="w_rot", bufs=16,
                                name=f"{dname}{t}")
                nc.sync.dma_start(tl[:], dram[dname].ap()[t])
                tiles.append(tl)
            return tiles

        # ---------------- Q projection:  qs = (Wq/8) @ qT + bq/8 ----------------
        qT = load_set("qT", SQ)
        wq = load_w("wq")
        for ot in range(NT):
            ps = psA.tile([128, S], F32, tag="psA", name=f"psq{ot}")
            for dt in range(NT):
                nc.tensor.matmul(ps[:, :SQ], wq[dt][:, ot * 128:(ot + 1) * 128],
                                 qT[dt][:, :SQ], start=(dt == 0),
                                 stop=(dt == NT - 1))
            nc.scalar.activation(qs[ot][:], ps[:, :SQ], Identity,
                                 bias=bq_sb[:, ot:ot + 1])

        # ---------------- K projection:  ks = Wk @ kT + bk ----------------
        kT = load_set("kT", S)
        wk = load_w("wk")
        for ot in range(NT):
            ps = psA.tile([128, S], F32, tag="psA", name=f"psk{ot}")
            for dt in range(NT):
                lhsT = wk[dt][:, ot * 128:(ot + 1) * 128]
                nc.tensor.matmul(ps[:, 0:512], lhsT, kT[dt][:, 0:512],
                                 start=(dt == 0), stop=(dt == NT - 1))
                nc.tensor.matmul(ps[:, 512:1024], lhsT, kT[dt][:, 512:1024],
                                 start=(dt == 0), stop=(dt == NT - 1))
            nc.scalar.activation(ks[ot][:], ps[:], Identity,
                                 bias=bk_sb[:, ot:ot + 1])

        # ------------- V projection (natural layout):  v = value @ Wv.T + bv ----
        vT = load_set("vT", S)
        wv = load_w("wv")
        for rt in range(NT):
            ps = psA.tile([128, S], F32, tag="psA", name=f"psv{rt}")
            for half in range(2):
                sl = slice(half * 512, (half + 1) * 512)
                for dt in range(NT):
                    nc.tensor.matmul(ps[:, sl], vT[dt][:, rt * 128:(rt + 1) * 128],
                                     wv[dt][:, sl], start=(dt == 0), stop=False)
                # bias via K=1 matmul: ones^T (1x128) x bv row (1x512)
                nc.tensor.matmul(ps[:, sl], ones1[:], bv_sb[:, sl],
                                 start=False, stop=True)
            nc.vector.tensor_copy(
                vsb[rt][:, :, 0:DK],
                ps[:].rearrange("p (h d) -> p h d", h=H))

        wo = load_w("wo")

        # ---------------- attention, one head pair at a time ----------------
        for p in range(NPAIR):
            # scores + exp + mask for both heads of the pair, per k-tile
            exps = []
            for kt in range(NT):
                ps = psA.tile([128, S], F32, tag="psA", name=f"pss{p}_{kt}")
                for i, h in enumerate((2 * p, 2 * p + 1)):
                    pbase = (h % 2) * 64
                    lhsT = ks[p][pbase:pbase + 64, kt * 128:(kt + 1) * 128]
                    rhs = qs[p][pbase:pbase + 64, :]
                    nc.tensor.matmul(ps[:, i * SQ:(i + 1) * SQ], lhsT, rhs,
                                     start=True, stop=True)
                ex = xpool.tile([128, 2, SQ], BF16, tag="expS", bufs=18,
                                name=f"ex{p}_{kt}")
                nc.scalar.activation(
                    ex[:], ps[:].rearrange("p (i q) -> p i q", i=2), Exp)
                nc.vector.tensor_tensor(
                    ex[:], ex[:],
                    mts[kt][:].unsqueeze(1).to_broadcast((128, 2, SQ)),
                    mybir.AluOpType.mult)
                exps.append(ex)

            # AV matmuls (M=65; ones column makes row 64 the denominator)
            av_tiles = []
            recips = []
            for i, h in enumerate((2 * p, 2 * p + 1)):
                ps = psB.tile([65, SQ], F32, tag="psB", bufs=4, name=f"psav{h}")
                for kt in range(NT):
                    nc.tensor.matmul(ps[:], vsb[kt][:, h, :], exps[kt][:, i, :],
                                     start=(kt == 0), stop=(kt == NT - 1))
                # move denominator row down to partition 0 (aligned copy, then
                # DMA — the only engine allowed to move across partitions)
                dh = small.tile([65, SQ], F32, tag="den_h", bufs=4, name=f"dh{h}")
                nc.vector.tensor_copy(dh[64:65, :], ps[64:65, :])
                den_h = small.tile([1, SQ], F32, tag="den0", bufs=4,
                                   name=f"den{h}")
                nc.sync.dma_start(den_h[:], dh[64:65, :])
                recip_h = small.tile([1, SQ], F32, tag="recip0", bufs=4,
                                     name=f"recip{h}")
                scr_h = small.tile([1, SQ], F32, tag="scr0", bufs=4,
                                   name=f"scr{h}")
                nc.vector.reciprocal_approx_accurate(recip_h[:], den_h[:],
                                                     scr_h[:])
                av_tiles.append(ps)
                recips.append(recip_h)

            # normalize + evict into attnT (odd head needs a partition-move DMA)
            for i, h in enumerate((2 * p, 2 * p + 1)):
                ps = av_tiles[i]
                bc = small.tile([64, SQ], F32, tag="bcast", bufs=4, name=f"bc{h}")
                nc.gpsimd.partition_broadcast(bc[:], recips[i][:])
                if i == 0:
                    nc.vector.tensor_tensor(attnT[p][0:64, :], ps[0:64, :],
                                            bc[:], mybir.AluOpType.mult)
                else:
                    st = small.tile([64, SQ], BF16, tag="stg", bufs=2,
                                    name=f"stg{h}")
                    nc.vector.tensor_tensor(st[:], ps[0:64, :], bc[:],
                                            mybir.AluOpType.mult)
                    nc.sync.dma_start(attnT[p][64:128, :], st[:])

        # ---------------- output projection ----------------
        for ot in range(NT):
            ps = psB.tile([128, SQ], F32, tag="psB", bufs=4, name=f"pso{ot}")
            for dt in range(NT):
                nc.tensor.matmul(ps[:], wo[dt][:, ot * 128:(ot + 1) * 128],
                                 attnT[dt][:], start=(dt == 0),
                                 stop=(dt == NT - 1))
            osb = opool.tile([128, SQ], F32, tag="osb", bufs=2, name=f"osb{ot}")
            nc.scalar.activation(osb[:], ps[:], Identity,
                                 bias=bo_sb[:, ot:ot + 1])
            nc.sync.dma_start(outT.ap()[ot], osb[:])


def _build():
    nc = bacc.Bacc("TRN2", target_bir_lowering=False, debug=False,
                   num_devices=NCORES)
    with tile.TileContext(nc) as tc:
        _emit(nc, tc)
    nc.compile()
    return nc


def _get_compiled():
    global _COMPILED
    if _COMPILED is None:
        _COMPILED = _build()
    return _COMPILED


def _tile3(x, dtype=ml_dtypes.bfloat16):
    # [D, N] -> [NT, 128, N] contiguous
    return np.ascontiguousarray(x.reshape(NT, 128, -1)).astype(dtype)


def kernel(**inputs):
    global LAST_RESULT
    query = np.asarray(inputs["query"], np.float32)
    key = np.asarray(inputs.get("key_in", inputs.get("key"))).astype(np.float32)
    value = np.asarray(inputs["value"], np.float32)
    mask = np.asarray(inputs["mask"])
    Wq = np.asarray(inputs["Wq"], np.float32)
    bq = np.asarray(inputs["bq"], np.float32)
    Wk = np.asarray(inputs["Wk"], np.float32)
    bk = np.asarray(inputs["bk"], np.float32)
    Wv = np.asarray(inputs["Wv"], np.float32)
    bv = np.asarray(inputs["bv"], np.float32)
    Wo = np.asarray(inputs["Wo"], np.float32)
    bo = np.asarray(inputs["bo"], np.float32)

    nc = _get_compiled()

    scale = np.float32(1.0 / np.sqrt(np.float32(DK)))
    shared = {
        "wq": _tile3(Wq.T * scale),       # (Wq/8)^T, d on partitions
        "wk": _tile3(Wk.T),
        "wv": _tile3(Wv.T),
        "wo": _tile3(Wo.T),
        "bqc": np.ascontiguousarray((bq * scale).reshape(NT, 128).T),
        "bkc": np.ascontiguousarray(bk.reshape(NT, 128).T),
        "boc": np.ascontiguousarray(bo.reshape(NT, 128).T),
        "bvr": bv.reshape(1, D).astype(ml_dtypes.bfloat16),
    }

    in_maps = []
    for c in range(NCORES):
        b, half = divmod(c, 2)
        qsl = slice(half * SQ, (half + 1) * SQ)
        m = dict(shared)
        m["qT"] = _tile3(query[b, qsl].T)
        m["kT"] = _tile3(key[b].T)
        m["vT"] = _tile3(value[b].T)
        m["mT"] = _tile3(mask[b, 0, qsl].T.astype(np.float32))
        in_maps.append(m)

    kwargs = {}
    if TRACE:
        kwargs = dict(trace=True, trace_cores=list(TRACE_CORES))
    res = bass_utils.run_bass_kernel_spmd(nc, in_maps,
                                          core_ids=list(range(NCORES)),
                                          **kwargs)
    LAST_RESULT = res

    out = np.empty((B, S, D), np.float32)
    for c in range(NCORES):
        b, half = divmod(c, 2)
        qsl = slice(half * SQ, (half + 1) * SQ)
        oT = res.results[c]["outT"].reshape(D, SQ)
        out[b, qsl] = oT.T
    return out


# revision 12
# speedup vs baseline: 1.0719x; 1.0719x over previous
"""Multi-head attention (B=4, S=1024, D=1024, H=16) on 8 Trainium2 NeuronCores.

Sharding: core c handles batch b = c//2 and query-half q = c%2 (512 query
rows).  Each core computes K/V projections for its batch's full sequence
(duplicated across the 2 cores sharing a batch), Q projection + attention +
output projection for its 512 query rows.  The full output is a pure
concatenation of the per-core outputs — no collectives needed.

On-chip layout keeps feature dims on SBUF partitions and token dims on the
free axis ("transposed" activations):
  - projections are matmuls with pre-transposed weights as the stationary
    operand, producing X.T layouts directly;
  - attention scores come out as S^T [k, q] (k on partitions) so the AV
    matmul needs no transposes at all;
  - softmax denominators come from an extra all-ones column appended to V
    (AV matmul with M=65: rows 0..63 = numerator, row 64 = denominator);
  - softmax is exp(score/8)*mask with no max subtraction (scores are O(1),
    and exp(-65500) == 0 exactly in fp32, so this matches the reference).
"""

import numpy as np
import ml_dtypes

import concourse.bass as bass
import concourse.tile as tile
from concourse import bacc, mybir
from concourse import bass_utils

B, S, D, H, DK = 4, 1024, 1024, 16, 64
SQ = S // 2            # query rows per core
NT = D // 128          # 8 partition tiles of the feature dim
NCORES = 8
NPAIR = H // 2         # head pairs (2 heads per 128-partition tile)
BF16 = mybir.dt.bfloat16
F32 = mybir.dt.float32
Exp = mybir.ActivationFunctionType.Exp
Identity = mybir.ActivationFunctionType.Identity

_COMPILED = None
TRACE = False
TRACE_CORES = [0]
LAST_RESULT = None


def _emit(nc, tc):
    dram = {n: nc.dram_tensor(n, shp, dt, kind="ExternalInput") for n, shp, dt in [
        ("qT", (NT, 128, SQ), BF16),
        ("kT", (NT, 128, S), BF16),
        ("vT", (NT, 128, S), BF16),
        ("mT", (NT, 128, SQ), BF16),
        ("wq", (NT, 128, D), BF16),
        ("wk", (NT, 128, D), BF16),
        ("wv", (NT, 128, D), BF16),
        ("wo", (NT, 128, D), BF16),
        ("bqc", (128, NT), F32),
        ("bkc", (128, NT), F32),
        ("boc", (128, NT), F32),
        ("bvr", (1, D), BF16),
    ]}
    outT = nc.dram_tensor("outT", (NT, 128, SQ), F32, kind="ExternalOutput")

    import contextlib
    stack = contextlib.ExitStack()
    with stack:
        wpool = stack.enter_context(tc.tile_pool(name="wpool", bufs=1))
        inpool = stack.enter_context(tc.tile_pool(name="inpool", bufs=1))
        acts = stack.enter_context(tc.tile_pool(name="acts", bufs=1))
        xpool = stack.enter_context(tc.tile_pool(name="xpool", bufs=1))
        small = stack.enter_context(tc.tile_pool(name="small", bufs=1))
        opool = stack.enter_context(tc.tile_pool(name="opool", bufs=1))
        psS = stack.enter_context(tc.tile_pool(name="psS", bufs=3, space="PSUM"))
        psU = stack.enter_context(tc.tile_pool(name="psU", bufs=2, space="PSUM"))

        # ---- persistent SBUF tiles ----
        qs = [acts.tile([128, SQ], BF16, name=f"qs{t}") for t in range(NT)]
        ks = [acts.tile([128, S], BF16, name=f"ks{t}") for t in range(NT)]
        vsb = [acts.tile([128, H, DK + 1], BF16, name=f"vsb{t}") for t in range(NT)]
        attnT = [acts.tile([128, SQ], BF16, name=f"attnT{t}") for t in range(NT)]
        mts = [acts.tile([128, SQ], BF16, name=f"mts{t}") for t in range(NT)]

        bq_sb = small.tile([128, NT], F32, name="bq_sb")
        bk_sb = small.tile([128, NT], F32, name="bk_sb")
        bo_sb = small.tile([128, NT], F32, name="bo_sb")
        bv_sb = small.tile([1, D], BF16, name="bv_sb")
        ones1 = small.tile([1, 128], BF16, name="ones1")

        nc.vector.memset(ones1[:], 1.0)
        for t in range(NT):
            nc.vector.memset(vsb[t][:, :, DK:DK + 1], 1.0)

        # ---- constant / bias loads ----
        nc.sync.dma_start(bq_sb[:], dram["bqc"].ap())
        nc.sync.dma_start(bk_sb[:], dram["bkc"].ap())
        nc.sync.dma_start(bo_sb[:], dram["boc"].ap())
        nc.sync.dma_start(bv_sb[:], dram["bvr"].ap())
        for t in range(NT):
            mt_t = mts[t]
            nc.sync.dma_start(mt_t[:], dram["mT"].ap()[t])

        # ---- input/weight tiles (rotating pools) ----
        # inpool: vT(8) + qT(8) live together; kT recycles vT's slots.
        # wpool: wv/wq/wo distinct; wk recycles wv's slots.
        def load_set(dname, n_free):
            tiles = []
            for t in range(NT):
                tl = inpool.tile([128, S], BF16, tag="in_rot", bufs=16,
                                 name=f"{dname}{t}")
                nc.sync.dma_start(tl[:, :n_free], dram[dname].ap()[t])
                tiles.append(tl)
            return tiles

        def load_w(dname, rot=False):
            tiles = []
            for t in range(NT):
                if rot:
                    tl = wpool.tile([128, D], BF16, tag="w_rot", bufs=8,
                                    name=f"{dname}{t}")
                else:
                    tl = wpool.tile([128, D], BF16, name=f"{dname}{t}")
                nc.sync.dma_start(tl[:], dram[dname].ap()[t])
                tiles.append(tl)
            return tiles

        vT = load_set("vT", S)
        wv = load_w("wv", rot=True)
        qT = load_set("qT", SQ)
        wq = load_w("wq")
        wo = load_w("wo")

        # ------------- V projection (natural layout):  v = value @ Wv.T + bv ----
        # Runs first: a long dense matmul stream that warms the PE clock (HAM)
        # while the remaining inputs stream in.
        for rt in range(NT):
            for half in range(2):
                sl = slice(half * 512, (half + 1) * 512)
                ps = psU.tile([128, SQ], F32, tag="psU", name=f"psv{rt}_{half}")
                for dt in range(NT):
                    nc.tensor.matmul(ps[:], vT[dt][:, rt * 128:(rt + 1) * 128],
                                     wv[dt][:, sl], start=(dt == 0), stop=False)
                # bias via K=1 matmul: ones^T (1x128) x bv row (1x512)
                nc.tensor.matmul(ps[:], ones1[:], bv_sb[:, sl],
                                 start=False, stop=True)
                nc.vector.tensor_copy(
                    vsb[rt][:, half * 8:(half + 1) * 8, 0:DK],
                    ps[:].rearrange("p (h d) -> p h d", h=8))

        kT = load_set("kT", S)      # reuses vT's slots
        wk = load_w("wk", rot=True)  # reuses wv's slots

        # ------------- per head pair: Q-proj, K-proj, scores, AV ----------------
        for p in range(NPAIR):
            # Q projection for this pair's o-tile
            ps = psU.tile([128, SQ], F32, tag="psU", name=f"psq{p}")
            for dt in range(NT):
                nc.tensor.matmul(ps[:], wq[dt][:, p * 128:(p + 1) * 128],
                                 qT[dt][:, :SQ], start=(dt == 0),
                                 stop=(dt == NT - 1))
            nc.scalar.activation(qs[p][:], ps[:], Identity,
                                 bias=bq_sb[:, p:p + 1])

            # K projection for this pair's o-tile (two half groups)
            for half in range(2):
                sl = slice(half * 512, (half + 1) * 512)
                ps = psU.tile([128, SQ], F32, tag="psU", name=f"psk{p}_{half}")
                for dt in range(NT):
                    nc.tensor.matmul(ps[:], wk[dt][:, p * 128:(p + 1) * 128],
                                     kT[dt][:, sl], start=(dt == 0),
                                     stop=(dt == NT - 1))
                nc.scalar.activation(ks[p][:, sl], ps[:], Identity,
                                     bias=bk_sb[:, p:p + 1])

            # scores + exp + mask per k-tile, with the AV matmuls of this pair
            # interleaved two k-tiles behind (so exp+mask are already done).
            # Both heads' AV accumulators live in one 2-bank psS-shaped tile
            # (same partitions, different banks).
            exps = []
            avt = psS.tile([128, S], F32, tag="psS", name=f"psav{p}")
            avps = [avt[0:65, 0:SQ], avt[0:65, SQ:2 * SQ]]

            def av_mms(kt):
                for i in range(2):
                    nc.tensor.matmul(avps[i], vsb[kt][:, 2 * p + i, :],
                                     exps[kt][:, i, :],
                                     start=(kt == 0), stop=(kt == NT - 1))

            for kt in range(NT):
                ps = psS.tile([128, S], F32, tag="psS", name=f"pss{p}_{kt}")
                for i, h in enumerate((2 * p, 2 * p + 1)):
                    pbase = (h % 2) * 64
                    lhsT = ks[p][pbase:pbase + 64, kt * 128:(kt + 1) * 128]
                    rhs = qs[p][pbase:pbase + 64, :]
                    nc.tensor.matmul(ps[:, i * SQ:(i + 1) * SQ], lhsT, rhs,
                                     start=True, stop=True)
                ex = xpool.tile([128, 2, SQ], BF16, tag="expS", bufs=12,
                                name=f"ex{p}_{kt}")
                nc.scalar.activation(
                    ex[:], ps[:].rearrange("p (i q) -> p i q", i=2), Exp)
                nc.vector.tensor_tensor(
                    ex[:], ex[:],
                    mts[kt][:].unsqueeze(1).to_broadcast((128, 2, SQ)),
                    mybir.AluOpType.mult)
                exps.append(ex)
                if kt >= 2:
                    av_mms(kt - 2)
            av_mms(NT - 2)
            av_mms(NT - 1)

            # softmax denominators: reciprocal of row 64 (stays on partition
            # 64 — engines cannot shift partitions), then GPSIMD broadcast
            # down to partitions 0..63, then normalize + evict into attnT.
            for i, h in enumerate((2 * p, 2 * p + 1)):
                ps = avps[i]
                dh = small.tile([65, SQ], F32, tag="den_h", bufs=4, name=f"dh{h}")
                nc.vector.tensor_copy(dh[64:65, :], ps[64:65, :])
                den0 = small.tile([1, SQ], F32, tag="den0", bufs=4,
                                  name=f"den{h}")
                nc.sync.dma_start(den0[:], dh[64:65, :])
                recip_h = small.tile([1, SQ], F32, tag="recip0", bufs=4,
                                     name=f"recip{h}")
                nc.vector.reciprocal_approx_fast(recip_h[:], den0[:])
                bc = small.tile([64, SQ], F32, tag="bcast", bufs=4, name=f"bc{h}")
                nc.gpsimd.partition_broadcast(bc[:], recip_h[:])
                if i == 0:
                    nc.vector.tensor_tensor(attnT[p][0:64, :], ps[0:64, :],
                                            bc[:], mybir.AluOpType.mult)
                else:
                    st = small.tile([64, SQ], BF16, tag="stg", bufs=2,
                                    name=f"stg{h}")
                    nc.vector.tensor_tensor(st[:], ps[0:64, :], bc[:],
                                            mybir.AluOpType.mult)
                    nc.sync.dma_start(attnT[p][64:128, :], st[:])

        # ---------------- output projection ----------------
        for ot in range(NT):
            ps = psU.tile([128, SQ], F32, tag="psU", name=f"pso{ot}")
            for dt in range(NT):
                nc.tensor.matmul(ps[:], wo[dt][:, ot * 128:(ot + 1) * 128],
                                 attnT[dt][:], start=(dt == 0),
                                 stop=(dt == NT - 1))
            osb = opool.tile([128, SQ], F32, tag="osb", bufs=2, name=f"osb{ot}")
            nc.scalar.activation(osb[:], ps[:], Identity,
                                 bias=bo_sb[:, ot:ot + 1])
            nc.sync.dma_start(outT.ap()[ot], osb[:])


def _build():
    nc = bacc.Bacc("TRN2", target_bir_lowering=False, debug=False,
                   num_devices=NCORES)
    with tile.TileContext(nc) as tc:
        _emit(nc, tc)
    nc.compile()
    return nc


def _get_compiled():
    global _COMPILED
    if _COMPILED is None:
        _COMPILED = _build()
    return _COMPILED


def _tile3(x, dtype=ml_dtypes.bfloat16):
    # [D, N] -> [NT, 128, N] contiguous
    return np.ascontiguousarray(x.reshape(NT, 128, -1)).astype(dtype)


def kernel(**inputs):
    global LAST_RESULT
    query = np.asarray(inputs["query"], np.float32)
    key = np.asarray(inputs.get("key_in", inputs.get("key"))).astype(np.float32)
    value = np.asarray(inputs["value"], np.float32)
    mask = np.asarray(inputs["mask"])
    Wq = np.asarray(inputs["Wq"], np.float32)
    bq = np.asarray(inputs["bq"], np.float32)
    Wk = np.asarray(inputs["Wk"], np.float32)
    bk = np.asarray(inputs["bk"], np.float32)
    Wv = np.asarray(inputs["Wv"], np.float32)
    bv = np.asarray(inputs["bv"], np.float32)
    Wo = np.asarray(inputs["Wo"], np.float32)
    bo = np.asarray(inputs["bo"], np.float32)

    nc = _get_compiled()

    scale = np.float32(1.0 / np.sqrt(np.float32(DK)))
    shared = {
        "wq": _tile3(Wq.T * scale),       # (Wq/8)^T, d on partitions
        "wk": _tile3(Wk.T),
        "wv": _tile3(Wv.T),
        "wo": _tile3(Wo.T),
        "bqc": np.ascontiguousarray((bq * scale).reshape(NT, 128).T),
        "bkc": np.ascontiguousarray(bk.reshape(NT, 128).T),
        "boc": np.ascontiguousarray(bo.reshape(NT, 128).T),
        "bvr": bv.reshape(1, D).astype(ml_dtypes.bfloat16),
    }

    in_maps = []
    for c in range(NCORES):
        b, half = divmod(c, 2)
        qsl = slice(half * SQ, (half + 1) * SQ)
        m = dict(shared)
        m["qT"] = _tile3(query[b, qsl].T)
        m["kT"] = _tile3(key[b].T)
        m["vT"] = _tile3(value[b].T)
        m["mT"] = _tile3(mask[b, 0, qsl].T.astype(np.float32))
        in_maps.append(m)

    kwargs = {}
    if TRACE:
        kwargs = dict(trace=True, trace_cores=list(TRACE_CORES))
    res = bass_utils.run_bass_kernel_spmd(nc, in_maps,
                                          core_ids=list(range(NCORES)),
                                          **kwargs)
    LAST_RESULT = res

    out = np.empty((B, S, D), np.float32)
    for c in range(NCORES):
        b, half = divmod(c, 2)
        qsl = slice(half * SQ, (half + 1) * SQ)
        oT = res.results[c]["outT"].reshape(D, SQ)
        out[b, qsl] = oT.T
    return out


# revision 16
# speedup vs baseline: 1.1530x; 1.0757x over previous
"""Multi-head attention (B=4, S=1024, D=1024, H=16) on 8 Trainium2 NeuronCores.

Sharding: core c handles batch b = c//2 and query-half q = c%2 (512 query
rows).  Each core computes K/V projections for its batch's full sequence
(duplicated across the 2 cores sharing a batch), Q projection + attention +
output projection for its 512 query rows.  The full output is a pure
concatenation of the per-core outputs — no collectives needed.

On-chip layout keeps feature dims on SBUF partitions and token dims on the
free axis ("transposed" activations):
  - projections are matmuls with pre-transposed weights as the stationary
    operand, producing X.T layouts directly;
  - attention scores come out as S^T [k, q] (k on partitions) so the AV
    matmul needs no transposes at all;
  - softmax denominators come from an extra all-ones column appended to V
    (AV matmul with M=65: rows 0..63 = numerator, row 64 = denominator);
  - softmax is exp(score/8)*mask with no max subtraction (scores are O(1),
    and exp(-65500) == 0 exactly in fp32, so this matches the reference).

Pipeline: V-projection first (dense matmul stream that warms the PE clock
while remaining inputs load), then one head-pair at a time
[Q-proj -> K-proj -> scores/exp/mask with AV trailing 3 k-tiles], then the
output projection.  All inputs arrive as one large DMA per tensor set.
"""

import numpy as np
import ml_dtypes

import concourse.bass as bass
import concourse.tile as tile
from concourse import bacc, mybir
from concourse import bass_utils

B, S, D, H, DK = 4, 1024, 1024, 16, 64
SQ = S // 2            # query rows per core
NT = D // 128          # 8 partition tiles of the feature dim
NCORES = 8
NPAIR = H // 2         # head pairs (2 heads per 128-partition tile)
BF16 = mybir.dt.bfloat16
F32 = mybir.dt.float32
Exp = mybir.ActivationFunctionType.Exp
Identity = mybir.ActivationFunctionType.Identity

_COMPILED = None
TRACE = False
TRACE_CORES = [0]
LAST_RESULT = None


def _emit(nc, tc):
    dram = {n: nc.dram_tensor(n, shp, dt, kind="ExternalInput") for n, shp, dt in [
        ("qT", (NT, 128, SQ), BF16),
        ("kT", (NT, 128, S), BF16),
        ("vT", (NT, 128, S), BF16),
        ("mT", (NT, 128, SQ), BF16),
        ("wq", (NT, 128, D), BF16),
        ("wk", (NT, 128, D), BF16),
        ("wv", (NT, 128, D), BF16),
        ("wo", (NT, 128, D), BF16),
        ("bqc", (128, NT), F32),
        ("bkc", (128, NT), F32),
        ("boc", (128, NT), F32),
        ("bvr", (1, D), BF16),
    ]}
    outT = nc.dram_tensor("outT", (NT, 128, SQ), F32, kind="ExternalOutput")

    def big_load(pool, dname, nfree, tag, bufs):
        """One DMA for a whole [NT,128,nfree] set into a [128,NT,nfree] tile."""
        t = pool.tile([128, NT, nfree], BF16, tag=tag, bufs=bufs, name=dname)
        nc.sync.dma_start(t[:], dram[dname].ap().rearrange("t p f -> p t f"))
        return t

    import contextlib
    stack = contextlib.ExitStack()
    with stack:
        wpool = stack.enter_context(tc.tile_pool(name="wpool", bufs=1))
        inpool = stack.enter_context(tc.tile_pool(name="inpool", bufs=1))
        acts = stack.enter_context(tc.tile_pool(name="acts", bufs=1))
        xpool = stack.enter_context(tc.tile_pool(name="xpool", bufs=1))
        small = stack.enter_context(tc.tile_pool(name="small", bufs=1))
        opool = stack.enter_context(tc.tile_pool(name="opool", bufs=1))
        psS = stack.enter_context(tc.tile_pool(name="psS", bufs=2, space="PSUM"))
        psAV = stack.enter_context(tc.tile_pool(name="psAV", bufs=2, space="PSUM"))
        psU = stack.enter_context(tc.tile_pool(name="psU", bufs=2, space="PSUM"))

        # ---- persistent SBUF tiles ----
        qs = [acts.tile([128, SQ], BF16, name=f"qs{t}") for t in range(NT)]
        ks = [acts.tile([128, S], BF16, name=f"ks{t}") for t in range(NT)]
        vsb = [acts.tile([128, H, DK + 1], BF16, name=f"vsb{t}") for t in range(NT)]
        attnT = [acts.tile([128, SQ], BF16, name=f"attnT{t}") for t in range(NT)]

        bq_sb = small.tile([128, NT], F32, name="bq_sb")
        bk_sb = small.tile([128, NT], F32, name="bk_sb")
        bo_sb = small.tile([128, NT], F32, name="bo_sb")
        bv_sb = small.tile([1, D], BF16, name="bv_sb")
        ones1 = small.tile([1, 128], BF16, name="ones1")

        nc.vector.memset(ones1[:], 1.0)
        for t in range(NT):
            nc.vector.memset(vsb[t][:, :, DK:DK + 1], 1.0)

        # ---- loads: V-projection inputs first, then the rest ----
        nc.sync.dma_start(bv_sb[:], dram["bvr"].ap())
        vT = big_load(inpool, "vT", S, "in_big", 2)
        wv = big_load(wpool, "wv", D, "w_rot3", 3)
        qT = big_load(inpool, "qT", SQ, "in_q", 1)
        wq = big_load(wpool, "wq", D, "w_rot3", 3)
        nc.sync.dma_start(bq_sb[:], dram["bqc"].ap())
        nc.sync.dma_start(bk_sb[:], dram["bkc"].ap())
        nc.sync.dma_start(bo_sb[:], dram["boc"].ap())
        mts = big_load(acts, "mT", SQ, "mts", 1)

        # ------------- V projection (natural layout):  v = value @ Wv.T + bv ----
        for rt in range(NT):
            for half in range(2):
                sl = slice(half * 512, (half + 1) * 512)
                ps = psU.tile([128, SQ], F32, tag="psU", name=f"psv{rt}_{half}")
                for dt in range(NT):
                    nc.tensor.matmul(ps[:], vT[:, dt, rt * 128:(rt + 1) * 128],
                                     wv[:, dt, sl], start=(dt == 0), stop=False)
                nc.tensor.matmul(ps[:], ones1[:], bv_sb[:, sl],
                                 start=False, stop=True)
                nc.vector.tensor_copy(
                    vsb[rt][:, half * 8:(half + 1) * 8, 0:DK],
                    ps[:].rearrange("p (h d) -> p h d", h=8))

        kT = big_load(inpool, "kT", S, "in_big", 2)   # reuses vT's slot
        wk = big_load(wpool, "wk", D, "w_rot3", 3)
        wo = big_load(wpool, "wo", D, "w_rot3", 3)    # reuses wv's slot

        # ------------- per head pair: Q-proj, K-proj, scores, AV ----------------
        for p in range(NPAIR):
            # Q projection for this pair's o-tile
            ps = psU.tile([128, SQ], F32, tag="psU", name=f"psq{p}")
            for dt in range(NT):
                nc.tensor.matmul(ps[:], wq[:, dt, p * 128:(p + 1) * 128],
                                 qT[:, dt, :], start=(dt == 0),
                                 stop=(dt == NT - 1))
            nc.scalar.activation(qs[p][:], ps[:], Identity,
                                 bias=bq_sb[:, p:p + 1])

            # K projection for this pair's o-tile (two half groups)
            for half in range(2):
                sl = slice(half * 512, (half + 1) * 512)
                ps = psU.tile([128, SQ], F32, tag="psU", name=f"psk{p}_{half}")
                for dt in range(NT):
                    nc.tensor.matmul(ps[:], wk[:, dt, p * 128:(p + 1) * 128],
                                     kT[:, dt, sl], start=(dt == 0),
                                     stop=(dt == NT - 1))
                nc.scalar.activation(ks[p][:, sl], ps[:], Identity,
                                     bias=bk_sb[:, p:p + 1])

            # scores + exp + mask per k-tile; AV matmuls trail by 3 k-tiles
            exps = []
            avps = [psAV.tile([65, SQ], F32, tag="psAV", name=f"psav{2*p+i}")
                    for i in range(2)]

            def av_mms(kt):
                for i in range(2):
                    nc.tensor.matmul(avps[i][:], vsb[kt][:, 2 * p + i, :],
                                     exps[kt][:, i, :],
                                     start=(kt == 0), stop=(kt == NT - 1))

            for kt in range(NT):
                ps = psS.tile([128, S], F32, tag="psS", name=f"pss{p}_{kt}")
                for i, h in enumerate((2 * p, 2 * p + 1)):
                    pbase = (h % 2) * 64
                    lhsT = ks[p][pbase:pbase + 64, kt * 128:(kt + 1) * 128]
                    rhs = qs[p][pbase:pbase + 64, :]
                    nc.tensor.matmul(ps[:, i * SQ:(i + 1) * SQ], lhsT, rhs,
                                     start=True, stop=True)
                ex = xpool.tile([128, 2, SQ], BF16, tag="expS", bufs=12,
                                name=f"ex{p}_{kt}")
                nc.scalar.activation(
                    ex[:], ps[:].rearrange("p (i q) -> p i q", i=2), Exp)
                nc.vector.tensor_tensor(
                    ex[:], ex[:],
                    mts[:, kt, :].unsqueeze(1).to_broadcast((128, 2, SQ)),
                    mybir.AluOpType.mult)
                exps.append(ex)
                if kt >= 3:
                    av_mms(kt - 3)
            for kt in range(NT - 3, NT):
                av_mms(kt)

            # softmax denominators: reciprocal of PSUM row 64 in place (base
            # 64 -> 64 is legal), one tiny DMA to partition 0 (only DMA may
            # move data across partitions), GPSIMD broadcast, normalize.
            for i, h in enumerate((2 * p, 2 * p + 1)):
                ps = avps[i]
                dh = small.tile([65, SQ], F32, tag="den_h", bufs=2, name=f"dh{h}")
                nc.vector.tensor_copy(dh[64:65, :], ps[64:65, :])
                den0 = small.tile([1, SQ], F32, tag="den0", bufs=2,
                                  name=f"den{h}")
                nc.sync.dma_start(den0[:], dh[64:65, :])
                recip0 = small.tile([1, SQ], F32, tag="recip0", bufs=2,
                                    name=f"recip0_{h}")
                nc.vector.reciprocal_approx_fast(recip0[:], den0[:])
                bc = small.tile([64, SQ], F32, tag="bcast", bufs=3, name=f"bc{h}")
                nc.gpsimd.partition_broadcast(bc[:], recip0[:])
                if i == 0:
                    nc.vector.tensor_tensor(attnT[p][0:64, :], ps[0:64, :],
                                            bc[:], mybir.AluOpType.mult)
                else:
                    st = small.tile([64, SQ], BF16, tag="stg", bufs=2,
                                    name=f"stg{h}")
                    nc.vector.tensor_tensor(st[:], ps[0:64, :], bc[:],
                                            mybir.AluOpType.mult)
                    nc.sync.dma_start(attnT[p][64:128, :], st[:])

        # ---------------- output projection ----------------
        for ot in range(NT):
            ps = psU.tile([128, SQ], F32, tag="psU", name=f"pso{ot}")
            for dt in range(NT):
                nc.tensor.matmul(ps[:], wo[:, dt, ot * 128:(ot + 1) * 128],
                                 attnT[dt][:], start=(dt == 0),
                                 stop=(dt == NT - 1))
            osb = opool.tile([128, SQ], F32, tag="osb", bufs=2, name=f"osb{ot}")
            nc.scalar.activation(osb[:], ps[:], Identity,
                                 bias=bo_sb[:, ot:ot + 1])
            nc.sync.dma_start(outT.ap()[ot], osb[:])


def _build():
    nc = bacc.Bacc("TRN2", target_bir_lowering=False, debug=False,
                   num_devices=NCORES)
    with tile.TileContext(nc) as tc:
        _emit(nc, tc)
    nc.compile()
    return nc


def _get_compiled():
    global _COMPILED
    if _COMPILED is None:
        _COMPILED = _build()
    return _COMPILED


def _tile3(x, dtype=ml_dtypes.bfloat16):
    # [D, N] -> [NT, 128, N] contiguous
    return np.ascontiguousarray(x.reshape(NT, 128, -1)).astype(dtype)


def kernel(**inputs):
    global LAST_RESULT
    query = np.asarray(inputs["query"], np.float32)
    key = np.asarray(inputs.get("key_in", inputs.get("key"))).astype(np.float32)
    value = np.asarray(inputs["value"], np.float32)
    mask = np.asarray(inputs["mask"])
    Wq = np.asarray(inputs["Wq"], np.float32)
    bq = np.asarray(inputs["bq"], np.float32)
    Wk = np.asarray(inputs["Wk"], np.float32)
    bk = np.asarray(inputs["bk"], np.float32)
    Wv = np.asarray(inputs["Wv"], np.float32)
    bv = np.asarray(inputs["bv"], np.float32)
    Wo = np.asarray(inputs["Wo"], np.float32)
    bo = np.asarray(inputs["bo"], np.float32)

    nc = _get_compiled()

    scale = np.float32(1.0 / np.sqrt(np.float32(DK)))
    shared = {
        "wq": _tile3(Wq.T * scale),       # (Wq/8)^T, d on partitions
        "wk": _tile3(Wk.T),
        "wv": _tile3(Wv.T),
        "wo": _tile3(Wo.T),
        "bqc": np.ascontiguousarray((bq * scale).reshape(NT, 128).T),
        "bkc": np.ascontiguousarray(bk.reshape(NT, 128).T),
        "boc": np.ascontiguousarray(bo.reshape(NT, 128).T),
        "bvr": bv.reshape(1, D).astype(ml_dtypes.bfloat16),
    }

    in_maps = []
    for c in range(NCORES):
        b, half = divmod(c, 2)
        qsl = slice(half * SQ, (half + 1) * SQ)
        m = dict(shared)
        m["qT"] = _tile3(query[b, qsl].T)
        m["kT"] = _tile3(key[b].T)
        m["vT"] = _tile3(value[b].T)
        m["mT"] = _tile3(mask[b, 0, qsl].T.astype(np.float32))
        in_maps.append(m)

    kwargs = {}
    if TRACE:
        kwargs = dict(trace=True, trace_cores=list(TRACE_CORES))
    res = bass_utils.run_bass_kernel_spmd(nc, in_maps,
                                          core_ids=list(range(NCORES)),
                                          **kwargs)
    LAST_RESULT = res

    out = np.empty((B, S, D), np.float32)
    for c in range(NCORES):
        b, half = divmod(c, 2)
        qsl = slice(half * SQ, (half + 1) * SQ)
        oT = res.results[c]["outT"].reshape(D, SQ)
        out[b, qsl] = oT.T
    return out
